# revision 8
# baseline (speedup 1.0000x reference)
"""Trainium2 Bass kernel for nn_NavigationNet.

Math notes (validated host-side vs the jax reference, rel-l2 ~3e-6):

- `teom` is never used by the network (the conv path is dead code): the
  teom-LSTM consumes a hard-zero input with zero initial state, so its
  hidden-state trajectory is identical for every batch element.  Its entire
  contribution reduces to a precomputable per-step bias vector, which we fold
  through Wtf/Wmix/W1 into a [32, 64] table `BC` feeding the first MLP layer.
- `c_out` is exactly zeros.
- The mix head `W1 @ (Wmix[:,64:] @ (Wof @ h))` collapses into one [32,64]
  matrix `Wc`.
- All four LSTM gates use a single sigmoid activation per step: the g-gate
  pre-activation rows are pre-scaled by 2 so tanh(g) = 2*sigmoid(2g) - 1,
  and the cell state is tracked as c/2 so the trailing *2 folds into the
  Tanh activation's free `scale` operand.
- The y feedback `y_t = o_t + 1.5*y_{t-1} - 0.5*y_{t-3}` is kept in a
  [2, 67*128] ring (3 seed slots hold obsv[:, 5:8]).

Sharding: pure data parallel, batch 1024 -> 8 cores x 128.  Per core the
layout is feature-on-partition, batch-on-free (128 batch lanes in the free
dimension), so every op in the recurrence is a small [P<=128, 128] tile op.
"""

import numpy as np

import concourse.bass as bass
import concourse.mybir as mybir
import concourse.tile as tile
from concourse.bass_utils import run_bass_kernel_spmd
from concourse.vector_clock import ScopedClock

F32 = mybir.dt.float32
AF = mybir.ActivationFunctionType
ALU = mybir.AluOpType

BS, OBS_LEN, TS, HID = 1024, 8, 64, 64
N_CORES = 8
BSH = BS // N_CORES  # 128 batch per core


# ---------------------------------------------------------------------------
# walrus in this container rejects >1 sync-wait per instruction; split them.
class _TileContextSW(tile.TileContext):
    def _drain_and_barrier(self, tick_clock, wait_clock):
        probe = self.nc.sync.nop(hint="wait_probe", nofuse=True)
        wait_clock.add_sem_waits(
            probe.ins, ScopedClock({None: tick_clock.global_clock})
        )
        si = probe.ins.sync_info
        waits = list(si.on_wait) if si is not None and si.on_wait else []
        probe.ins.sync_info = None
        for w in waits:
            n = self.nc.sync.nop(hint="split_wait", nofuse=True)
            n.ins.sync_info = mybir.SyncInfo(on_wait=[w], on_update=[])
        self.nc.sync.drain()
        self.nc.all_engine_barrier()
        assert self.sems is not None
        popped = self.nc._tile_sem_poison_stack.pop()
        assert popped is self._sem_poison
        self.nc.clear_and_free_semaphores(list(self.sems.allocated().values()))
        self.nc.all_engine_barrier()


def _split_multi_waits(nc):
    for fn in nc.m.functions:
        for blk in fn.blocks:
            insts = blk.instructions
            out = []
            changed = False
            for inst in insts:
                si = inst.sync_info
                waits = list(si.on_wait) if si is not None and si.on_wait else []
                if len(waits) > 1:
                    for j, w in enumerate(waits[:-1]):
                        out.append(
                            mybir.InstNoOp(
                                name=f"{inst.name}-sw{j}",
                                engine=inst.engine,
                                ins=[],
                                outs=[],
                                sync_info=mybir.SyncInfo(on_wait=[w], on_update=[]),
                                bass_nofuse=True,
                            )
                        )
                    inst.sync_info = mybir.SyncInfo(
                        on_wait=[waits[-1]],
                        on_update=list(si.on_update) if si.on_update else [],
                    )
                    changed = True
                out.append(inst)
            if changed:
                insts[:] = out


# ---------------------------------------------------------------------------
# Host-side constant folding (float32 throughout, like the reference).
def _host_tables(W):
    f32 = np.float32
    sig = lambda v: (1.0 / (1.0 + np.exp(-v, dtype=np.float64))).astype(f32)
    tanh = lambda v: np.tanh(v.astype(np.float64)).astype(f32)

    b_o = W["bih_o"] + W["bhh_o"]
    b_t = W["bih_t"] + W["bhh_t"]
    Wmix_t = W["Wmix"][:, :64]
    Wmix_o = W["Wmix"][:, 64:]
    base = Wmix_o @ W["bof"] + W["bmix"]

    h = np.zeros(HID, f32)
    c = np.zeros(HID, f32)
    BC = np.zeros((32, TS), f32)
    for t in range(TS):
        g = W["Whh_t"] @ h + b_t
        i, f, gg, o = g[:64], g[64:128], g[128:192], g[192:256]
        c = sig(f) * c + sig(i) * tanh(gg)
        h = sig(o) * tanh(c)
        ts_ = W["Wtf"] @ h + W["btf"]
        BC[:, t] = W["W1"] @ (Wmix_t @ ts_ + base) + W["b1"]
    Wc = W["W1"] @ Wmix_o @ W["Wof"]

    Wt = np.concatenate([W["Whh_o"], b_o[:, None]], axis=1)  # [256, 65]
    Wx = W["Wih_o"].copy()  # [256, 2]
    Wt = Wt.copy()
    Wt[128:192] *= 2.0  # g-gate rows via tanh(x) = 2 sig(2x) - 1
    Wx[128:192] *= 2.0
    # gate-row order [f; i; o; g2]: keeps every later tensor-tensor pair on
    # matching base partitions (f/o at 0, i/g at 64).
    perm = np.concatenate([
        np.arange(64, 128), np.arange(0, 64),
        np.arange(192, 256), np.arange(128, 192)])
    Wt = Wt[perm]
    Wx = Wx[perm]

    return dict(
        lhsT_h=np.ascontiguousarray(Wt.T),          # [65, 256]
        lhsT_x=np.ascontiguousarray(Wx.T),          # [2, 256]
        lhsT_wc=np.ascontiguousarray(Wc.T),         # [64, 32]
        lhsT_w2=np.ascontiguousarray(W["W2"].T),    # [32, 32]
        lhsT_w3=np.ascontiguousarray(W["W3"].T),    # [32, 2]
        BCT=BC,                                     # [32, 64]
        b2c=np.ascontiguousarray(W["b2"][:, None]),  # [32, 1]
        b3=W["b3"],                                  # [2]
        ident2=np.eye(2, dtype=f32),                 # [2, 2]
    )


# ---------------------------------------------------------------------------
def _build_bass():
    nc = bass.Bass()

    din = {}
    for name, shape in [
        ("obsT", [2, OBS_LEN * BSH]),
        ("ys0", [2, 3 * BSH]),
        ("w0", [2, BSH]),
        ("b3b", [2, BSH]),
        ("lhsT_h", [65, 256]),
        ("lhsT_x", [2, 256]),
        ("lhsT_wc", [64, 32]),
        ("lhsT_w2", [32, 32]),
        ("lhsT_w3", [32, 2]),
        ("BCT", [32, TS]),
        ("b2c", [32, 1]),
        ("ident2", [2, 2]),
    ]:
        din[name] = nc.dram_tensor(name, shape, F32, kind="ExternalInput")
    y_out = nc.dram_tensor("y", [BSH, 2 * TS], F32, kind="ExternalOutput")

    with _TileContextSW(nc) as tc:
        with (
            tc.tile_pool(name="consts", bufs=1) as cp,
            tc.tile_pool(name="work", bufs=2) as wk,
            tc.tile_pool(name="pg", bufs=2, space="PSUM") as pg,
            tc.tile_pool(name="pmix", bufs=2, space="PSUM") as pmix,
            tc.tile_pool(name="pyp", bufs=1, space="PSUM") as pyp,
            tc.tile_pool(name="pout", bufs=1, space="PSUM") as pout,
        ):
            # --- constants into SBUF
            sb = {}
            for name, t in din.items():
                sb[name] = cp.tile(list(t.shape), F32, tag=name, name=name)
                nc.sync.dma_start(out=sb[name], in_=t[:, :])

            hx = cp.tile([65, BSH], F32, tag="hx")     # h (0:64) + ones row (64)
            ch = cp.tile([64, BSH], F32, tag="ch")     # c/2
            ys = cp.tile([2, (TS + 3) * BSH], F32, tag="ys")
            O_sb = cp.tile([BSH, 2 * TS], F32, tag="O_sb")

            nc.vector.memset(hx[0:64, :], 0.0)
            nc.vector.memset(hx[64:65, :], 1.0)
            nc.vector.memset(ch, 0.0)
            nc.sync.dma_start(out=ys[:, 0 : 3 * BSH], in_=din["ys0"][:, :])

            O_ps = pout.tile([BSH, 2 * TS], F32, tag="O_ps")

            lh, lx = sb["lhsT_h"], sb["lhsT_x"]
            obsT = sb["obsT"]

            def cell(x_ap):
                """One obsv-LSTM step; x_ap is the [2, BSH] input columns."""
                G = pg.tile([128, 256], F32, tag="G")
                nc.tensor.matmul(G[:, 0:128], lh[:, 0:128], hx,
                                 start=True, stop=False)
                nc.tensor.matmul(G[:, 0:128], lx[:, 0:128], x_ap,
                                 start=False, stop=True)
                nc.tensor.matmul(G[:, 128:256], lh[:, 128:256], hx,
                                 start=True, stop=False)
                nc.tensor.matmul(G[:, 128:256], lx[:, 128:256], x_ap,
                                 start=False, stop=True)
                S = wk.tile([128, 256], F32, tag="S")
                nc.scalar.activation(out=S, in_=G, func=AF.Sigmoid)
                S_f, S_i = S[0:64, 0:128], S[64:128, 0:128]
                S_o, S_g = S[0:64, 128:256], S[64:128, 128:256]
                tt = wk.tile([64, BSH], F32, tag="tt")
                nc.vector.scalar_tensor_tensor(
                    out=tt, in0=S_g, scalar=0.5, in1=S_i,
                    op0=ALU.subtract, op1=ALU.mult)
                m2 = wk.tile([64, BSH], F32, tag="m2")
                nc.gpsimd.tensor_mul(out=m2, in0=S_f, in1=ch)
                nc.vector.tensor_add(out=ch, in0=tt, in1=m2)
                th = wk.tile([64, BSH], F32, tag="th")
                nc.scalar.activation(out=th, in_=ch, func=AF.Tanh, scale=2.0)
                nc.vector.tensor_mul(out=hx[0:64, :], in0=S_o, in1=th)

            # --- warmup scan over the observed trajectory
            for k in range(OBS_LEN):
                cell(obsT[:, k * BSH : (k + 1) * BSH])

            # --- 64 output steps
            w_cur = cp.tile([2, BSH], F32, tag="w0_t")
            nc.sync.dma_start(out=w_cur, in_=din["w0"][:, :])

            for t in range(TS):
                if t > 0:
                    cell(ys[:, (t + 2) * BSH : (t + 3) * BSH])
                P1 = pmix.tile([32, BSH], F32, tag="P1")
                nc.tensor.matmul(P1, sb["lhsT_wc"], hx[0:64, :],
                                 start=True, stop=True)
                a1 = wk.tile([32, BSH], F32, tag="a1")
                nc.scalar.activation(out=a1, in_=P1, func=AF.Prelu,
                                     bias=sb["BCT"][:, t : t + 1], alpha=0.1)
                P2 = pmix.tile([32, BSH], F32, tag="P2")
                nc.tensor.matmul(P2, sb["lhsT_w2"], a1, start=True, stop=True)
                a2 = wk.tile([32, BSH], F32, tag="a2")
                nc.scalar.activation(out=a2, in_=P2, func=AF.Prelu,
                                     bias=sb["b2c"], alpha=0.1)
                PY = pyp.tile([2, BSH], F32, tag="PY")
                nc.tensor.matmul(PY, sb["lhsT_w3"], a2, start=True, stop=True)
                ys_s = ys[:, (t + 3) * BSH : (t + 4) * BSH]
                nc.vector.tensor_add(out=ys_s, in0=PY, in1=w_cur)
                # accumulate transposed output column pair
                nc.tensor.matmul(O_ps[:, 2 * t : 2 * t + 2], ys_s,
                                 sb["ident2"], is_transpose=True,
                                 start=True, stop=True)
                # w for step t+1 = 1.5*y_t - 0.5*y_{t-2} + b3  (off critical path)
                u = wk.tile([2, BSH], F32, tag="u")
                nc.vector.scalar_tensor_tensor(
                    out=u, in0=ys_s, scalar=1.5, in1=sb["b3b"],
                    op0=ALU.mult, op1=ALU.add)
                wn = wk.tile([2, BSH], F32, tag="wn")
                nc.vector.scalar_tensor_tensor(
                    out=wn, in0=ys[:, (t + 1) * BSH : (t + 2) * BSH],
                    scalar=-0.5, in1=u, op0=ALU.mult, op1=ALU.add)
                w_cur = wn

            nc.scalar.copy(out=O_sb, in_=O_ps)
            nc.sync.dma_start(out=y_out[:, :], in_=O_sb)

    _split_multi_waits(nc)
    return nc


_CACHED = {}


def _get_nc():
    if "nc" not in _CACHED:
        _CACHED["nc"] = _build_bass()
    return _CACHED["nc"]


def _in_maps(inputs):
    f32 = np.float32
    W = {k: np.ascontiguousarray(np.asarray(v), dtype=f32) for k, v in inputs.items()
         if k != "teom"}
    tabs = _host_tables(W)
    b3 = tabs.pop("b3")
    obsv = W["obsv"]  # [1024, 8, 2]

    const_part = {k: np.ascontiguousarray(v, dtype=f32) for k, v in tabs.items()}
    const_part["b3b"] = np.ascontiguousarray(
        np.broadcast_to(b3[:, None], (2, BSH)), dtype=f32)

    maps = []
    for i in range(N_CORES):
        sh = obsv[i * BSH : (i + 1) * BSH]  # [128, 8, 2]
        m = dict(const_part)
        m["obsT"] = np.ascontiguousarray(
            sh.transpose(2, 1, 0).reshape(2, OBS_LEN * BSH))
        m["ys0"] = np.ascontiguousarray(
            sh[:, 5:8, :].transpose(2, 1, 0).reshape(2, 3 * BSH))
        w0 = 1.5 * sh[:, 7, :] - 0.5 * sh[:, 5, :] + b3  # [128, 2]
        m["w0"] = np.ascontiguousarray(w0.T)
        maps.append(m)
    return maps


def run(inputs, trace=False, **kw):
    nc = _get_nc()
    res = run_bass_kernel_spmd(
        nc, _in_maps(inputs), core_ids=list(range(N_CORES)), trace=trace, **kw)
    ys = [r["y"].reshape(BSH, TS, 2) for r in res.results]
    y = np.concatenate(ys, axis=0)
    c = np.zeros((BS, TS), np.float32)
    return (y, c), res


def kernel(**inputs):
    (y, c), _ = run(inputs)
    return y, c


# revision 18
# speedup vs baseline: 1.5053x; 1.5053x over previous
"""Trainium2 Bass kernel for nn_NavigationNet.

Math notes (validated host-side vs the jax reference, rel-l2 ~3e-6):

- `teom` is never used by the network (the conv path is dead code): the
  teom-LSTM consumes a hard-zero input with zero initial state, so its
  hidden-state trajectory is identical for every batch element.  Its entire
  contribution reduces to a precomputable per-step bias vector, which we fold
  through Wtf/Wmix/W1 into a [32, 64] table `BC` feeding the first MLP layer.
- `c_out` is exactly zeros.
- The mix head `W1 @ (Wmix[:,64:] @ (Wof @ h))` collapses into one [32,64]
  matrix `Wc`.
- All four LSTM gates use a single sigmoid activation per step: the g-gate
  pre-activation rows are pre-scaled by 2 so tanh(g) = 2*sigmoid(2g) - 1,
  and the cell state is tracked as c/2 so the trailing *2 folds into the
  Tanh activation's free `scale` operand.
- The y feedback `y_t = o_t + 1.5*y_{t-1} - 0.5*y_{t-3}` is kept in a
  [2, 67*128] ring (3 seed slots hold obsv[:, 5:8]).

Sharding: pure data parallel, batch 1024 -> 8 cores x 128.  Per core the
layout is feature-on-partition, batch-on-free (128 batch lanes in the free
dimension), so every op in the recurrence is a small [P<=128, 128] tile op.
"""

import numpy as np

import concourse.bass as bass
import concourse.mybir as mybir
import concourse.tile as tile
from concourse.bass_utils import run_bass_kernel_spmd
from concourse.vector_clock import ScopedClock

F32 = mybir.dt.float32
BF16 = mybir.dt.bfloat16
AF = mybir.ActivationFunctionType
ALU = mybir.AluOpType

BS, OBS_LEN, TS, HID = 1024, 8, 64, 64
N_CORES = 8
BSH = BS // N_CORES  # 128 batch per core


# ---------------------------------------------------------------------------
# walrus in this container rejects >1 sync-wait per instruction; split them.
class _TileContextSW(tile.TileContext):
    def _drain_and_barrier(self, tick_clock, wait_clock):
        probe = self.nc.sync.nop(hint="wait_probe", nofuse=True)
        wait_clock.add_sem_waits(
            probe.ins, ScopedClock({None: tick_clock.global_clock})
        )
        si = probe.ins.sync_info
        waits = list(si.on_wait) if si is not None and si.on_wait else []
        probe.ins.sync_info = None
        for w in waits:
            n = self.nc.sync.nop(hint="split_wait", nofuse=True)
            n.ins.sync_info = mybir.SyncInfo(on_wait=[w], on_update=[])
        self.nc.sync.drain()
        self.nc.all_engine_barrier()
        assert self.sems is not None
        popped = self.nc._tile_sem_poison_stack.pop()
        assert popped is self._sem_poison
        self.nc.clear_and_free_semaphores(list(self.sems.allocated().values()))
        self.nc.all_engine_barrier()


def _split_multi_waits(nc):
    for fn in nc.m.functions:
        for blk in fn.blocks:
            insts = blk.instructions
            out = []
            changed = False
            for inst in insts:
                si = inst.sync_info
                waits = list(si.on_wait) if si is not None and si.on_wait else []
                if len(waits) > 1:
                    for j, w in enumerate(waits[:-1]):
                        out.append(
                            mybir.InstNoOp(
                                name=f"{inst.name}-sw{j}",
                                engine=inst.engine,
                                ins=[],
                                outs=[],
                                sync_info=mybir.SyncInfo(on_wait=[w], on_update=[]),
                                bass_nofuse=True,
                            )
                        )
                    inst.sync_info = mybir.SyncInfo(
                        on_wait=[waits[-1]],
                        on_update=list(si.on_update) if si.on_update else [],
                    )
                    changed = True
                out.append(inst)
            if changed:
                insts[:] = out


# ---------------------------------------------------------------------------
# Host-side constant folding (float32 throughout, like the reference).
def _host_tables(W):
    f32 = np.float32
    sig = lambda v: (1.0 / (1.0 + np.exp(-v, dtype=np.float64))).astype(f32)
    tanh = lambda v: np.tanh(v.astype(np.float64)).astype(f32)

    b_o = W["bih_o"] + W["bhh_o"]
    b_t = W["bih_t"] + W["bhh_t"]
    Wmix_t = W["Wmix"][:, :64]
    Wmix_o = W["Wmix"][:, 64:]
    base = Wmix_o @ W["bof"] + W["bmix"]

    h = np.zeros(HID, f32)
    c = np.zeros(HID, f32)
    BC = np.zeros((32, TS), f32)
    for t in range(TS):
        g = W["Whh_t"] @ h + b_t
        i, f, gg, o = g[:64], g[64:128], g[128:192], g[192:256]
        c = sig(f) * c + sig(i) * tanh(gg)
        h = sig(o) * tanh(c)
        ts_ = W["Wtf"] @ h + W["btf"]
        BC[:, t] = W["W1"] @ (Wmix_t @ ts_ + base) + W["b1"]
    Wc = W["W1"] @ Wmix_o @ W["Wof"]

    Wt = np.concatenate([W["Whh_o"], b_o[:, None]], axis=1)  # [256, 65]
    Wx = W["Wih_o"].copy()  # [256, 2]
    Wt = Wt.copy()
    Wt[128:192] *= 2.0  # g-gate rows via tanh(x) = 2 sig(2x) - 1
    Wx[128:192] *= 2.0
    # gate-row order [f; i; o; g2]: keeps every later tensor-tensor pair on
    # matching base partitions (f/o at 0, i/g at 64).
    perm = np.concatenate([
        np.arange(64, 128), np.arange(0, 64),
        np.arange(192, 256), np.arange(128, 192)])
    Wt = Wt[perm]
    Wx = Wx[perm]

    import ml_dtypes
    bf16 = ml_dtypes.bfloat16
    return dict(
        lhsT_h=np.ascontiguousarray(Wt.T.astype(bf16)),        # [65, 256]
        lhsT_x=np.ascontiguousarray(Wx.T.astype(bf16)),        # [2, 256]
        lhsT_wc=np.ascontiguousarray(Wc.T.astype(bf16)),       # [64, 32]
        lhsT_w2=np.ascontiguousarray(W["W2"].T.astype(bf16)),  # [32, 32]
        lhsT_w3=np.ascontiguousarray(W["W3"].T.astype(bf16)),  # [32, 2]
        BCT=BC,                                     # [32, 64]
        b2c=np.ascontiguousarray(W["b2"][:, None]),  # [32, 1]
        b3=W["b3"],                                  # [2]
        ident2=np.eye(2, dtype=f32),                 # [2, 2]
    )


# ---------------------------------------------------------------------------
def _build_bass():
    nc = bass.Bass()

    din = {}
    for name, shape, dt in [
        ("obsT", [2, OBS_LEN * BSH], BF16),
        ("ys0", [2, 3 * BSH], F32),
        ("w0", [2, BSH], F32),
        ("b3b", [2, BSH], F32),
        ("lhsT_h", [65, 256], BF16),
        ("lhsT_x", [2, 256], BF16),
        ("lhsT_wc", [64, 32], BF16),
        ("lhsT_w2", [32, 32], BF16),
        ("lhsT_w3", [32, 2], BF16),
        ("BCT", [32, TS], F32),
        ("b2c", [32, 1], F32),
        ("ident2", [2, 2], F32),
    ]:
        din[name] = nc.dram_tensor(name, shape, dt, kind="ExternalInput")
    y_out = nc.dram_tensor("y", [BSH, 2 * TS], F32, kind="ExternalOutput")

    with _TileContextSW(nc) as tc:
        with (
            tc.tile_pool(name="consts", bufs=1) as cp,
            tc.tile_pool(name="work", bufs=2) as wk,
            tc.tile_pool(name="pg", bufs=2, space="PSUM") as pg,
            tc.tile_pool(name="pmix", bufs=2, space="PSUM") as pmix,
            tc.tile_pool(name="pyp", bufs=1, space="PSUM") as pyp,
            tc.tile_pool(name="pout", bufs=1, space="PSUM") as pout,
        ):
            # --- constants into SBUF
            sb = {}
            for name, t in din.items():
                sb[name] = cp.tile(list(t.shape), t.dtype, tag=name, name=name)
                nc.sync.dma_start(out=sb[name], in_=t[:, :])

            hx = cp.tile([65, BSH], BF16, tag="hx")    # h (0:64) + ones row (64)
            ch = cp.tile([64, BSH], F32, tag="ch")     # c/2
            ys = cp.tile([2, (TS + 3) * BSH], F32, tag="ys")
            O_sb = cp.tile([BSH, 2 * TS], F32, tag="O_sb")

            nc.vector.memset(hx[0:64, :], 0.0)
            nc.vector.memset(hx[64:65, :], 1.0)
            nc.vector.memset(ch, 0.0)
            nc.sync.dma_start(out=ys[:, 0 : 3 * BSH], in_=din["ys0"][:, :])

            O_ps = pout.tile([BSH, 2 * TS], F32, tag="O_ps")

            lh, lx = sb["lhsT_h"], sb["lhsT_x"]
            obsT = sb["obsT"]

            def cell(x_ap):
                """One obsv-LSTM step; x_ap is the [2, BSH] input columns."""
                G = pg.tile([128, 256], F32, tag="G")
                nc.tensor.matmul(G[:, 0:128], lh[:, 0:128], hx,
                                 start=True, stop=False)
                nc.tensor.matmul(G[:, 0:128], lx[:, 0:128], x_ap,
                                 start=False, stop=True)
                nc.tensor.matmul(G[:, 128:256], lh[:, 128:256], hx,
                                 start=True, stop=False)
                nc.tensor.matmul(G[:, 128:256], lx[:, 128:256], x_ap,
                                 start=False, stop=True)
                S = wk.tile([128, 256], F32, tag="S")
                nc.scalar.activation(out=S, in_=G, func=AF.Sigmoid)
                S_f, S_i = S[0:64, 0:128], S[64:128, 0:128]
                S_o, S_g = S[0:64, 128:256], S[64:128, 128:256]
                tt = wk.tile([64, BSH], F32, tag="tt")
                nc.vector.scalar_tensor_tensor(
                    out=tt, in0=S_g, scalar=0.5, in1=S_i,
                    op0=ALU.subtract, op1=ALU.mult)
                m2 = wk.tile([64, BSH], F32, tag="m2")
                nc.vector.tensor_mul(out=m2, in0=S_f, in1=ch)
                nc.vector.tensor_add(out=ch, in0=tt, in1=m2)
                th = wk.tile([64, BSH], F32, tag="th")
                nc.scalar.activation(out=th, in_=ch, func=AF.Tanh, scale=2.0)
                nc.vector.tensor_mul(out=hx[0:64, :], in0=S_o, in1=th)

            # --- warmup scan over the observed trajectory
            for k in range(OBS_LEN):
                cell(obsT[:, k * BSH : (k + 1) * BSH])

            # --- 64 output steps
            w_cur = cp.tile([2, BSH], F32, tag="w0_t")
            nc.sync.dma_start(out=w_cur, in_=din["w0"][:, :])

            yb_prev = None
            for t in range(TS):
                if t > 0:
                    cell(yb_prev)
                P1 = pmix.tile([32, BSH], F32, tag="P1")
                nc.tensor.matmul(P1, sb["lhsT_wc"], hx[0:64, :],
                                 start=True, stop=True)
                a1 = wk.tile([32, BSH], BF16, tag="a1")
                nc.scalar.activation(out=a1, in_=P1, func=AF.Prelu,
                                     bias=sb["BCT"][:, t : t + 1], alpha=0.1)
                P2 = pmix.tile([32, BSH], F32, tag="P2")
                nc.tensor.matmul(P2, sb["lhsT_w2"], a1, start=True, stop=True)
                a2 = wk.tile([32, BSH], BF16, tag="a2")
                nc.scalar.activation(out=a2, in_=P2, func=AF.Prelu,
                                     bias=sb["b2c"], alpha=0.1)
                PY = pyp.tile([2, BSH], F32, tag="PY")
                nc.tensor.matmul(PY, sb["lhsT_w3"], a2, start=True, stop=True)
                ys_s = ys[:, (t + 3) * BSH : (t + 4) * BSH]
                nc.vector.tensor_add(out=ys_s, in0=PY, in1=w_cur)
                yb = wk.tile([2, BSH], BF16, tag="yb")
                nc.vector.tensor_copy(yb, ys_s)
                yb_prev = yb
                # accumulate transposed output column pair
                nc.tensor.matmul(O_ps[:, 2 * t : 2 * t + 2], ys_s,
                                 sb["ident2"], is_transpose=True,
                                 start=True, stop=True)
                # w for step t+1 = 1.5*y_t - 0.5*y_{t-2} + b3  (off critical path)
                u = wk.tile([2, BSH], F32, tag="u")
                nc.vector.scalar_tensor_tensor(
                    out=u, in0=ys_s, scalar=1.5, in1=sb["b3b"],
                    op0=ALU.mult, op1=ALU.add)
                wn = wk.tile([2, BSH], F32, tag="wn")
                nc.vector.scalar_tensor_tensor(
                    out=wn, in0=ys[:, (t + 1) * BSH : (t + 2) * BSH],
                    scalar=-0.5, in1=u, op0=ALU.mult, op1=ALU.add)
                w_cur = wn

            nc.scalar.copy(out=O_sb, in_=O_ps)
            nc.sync.dma_start(out=y_out[:, :], in_=O_sb)

    _split_multi_waits(nc)
    return nc


_CACHED = {}


def _get_nc():
    if "nc" not in _CACHED:
        _CACHED["nc"] = _build_bass()
    return _CACHED["nc"]


def _in_maps(inputs):
    f32 = np.float32
    W = {k: np.ascontiguousarray(np.asarray(v), dtype=f32) for k, v in inputs.items()
         if k != "teom"}
    tabs = _host_tables(W)
    b3 = tabs.pop("b3")
    obsv = W["obsv"]  # [1024, 8, 2]

    const_part = {k: np.ascontiguousarray(v) for k, v in tabs.items()}
    const_part["b3b"] = np.ascontiguousarray(
        np.broadcast_to(b3[:, None], (2, BSH)), dtype=f32)

    maps = []
    for i in range(N_CORES):
        sh = obsv[i * BSH : (i + 1) * BSH]  # [128, 8, 2]
        m = dict(const_part)
        import ml_dtypes
        m["obsT"] = np.ascontiguousarray(
            sh.transpose(2, 1, 0).reshape(2, OBS_LEN * BSH).astype(ml_dtypes.bfloat16))
        m["ys0"] = np.ascontiguousarray(
            sh[:, 5:8, :].transpose(2, 1, 0).reshape(2, 3 * BSH))
        w0 = 1.5 * sh[:, 7, :] - 0.5 * sh[:, 5, :] + b3  # [128, 2]
        m["w0"] = np.ascontiguousarray(w0.T)
        maps.append(m)
    return maps


def run(inputs, trace=False, **kw):
    nc = _get_nc()
    res = run_bass_kernel_spmd(
        nc, _in_maps(inputs), core_ids=list(range(N_CORES)), trace=trace, **kw)
    ys = [r["y"].reshape(BSH, TS, 2) for r in res.results]
    y = np.concatenate(ys, axis=0)
    c = np.zeros((BS, TS), np.float32)
    return (y, c), res


def kernel(**inputs):
    (y, c), _ = run(inputs)
    return y, c


# revision 22
# speedup vs baseline: 1.7620x; 1.1705x over previous
"""Trainium2 Bass kernel for nn_NavigationNet.

Math notes (validated host-side vs the jax reference, rel-l2 ~3e-6):

- `teom` is never used by the network (the conv path is dead code): the
  teom-LSTM consumes a hard-zero input with zero initial state, so its
  hidden-state trajectory is identical for every batch element.  Its entire
  contribution reduces to a precomputable per-step bias vector, which we fold
  through Wtf/Wmix/W1 into a [32, 64] table `BC` feeding the first MLP layer.
- `c_out` is exactly zeros.
- The mix head `W1 @ (Wmix[:,64:] @ (Wof @ h))` collapses into one [32,64]
  matrix `Wc`.
- All four LSTM gates use a single sigmoid activation per step: the g-gate
  pre-activation rows are pre-scaled by 2 so tanh(g) = 2*sigmoid(2g) - 1,
  and the cell state is tracked as c/2 so the trailing *2 folds into the
  Tanh activation's free `scale` operand.
- The y feedback `y_t = o_t + 1.5*y_{t-1} - 0.5*y_{t-3}` is kept in a
  [2, 67*128] ring (3 seed slots hold obsv[:, 5:8]).

Sharding: pure data parallel, batch 1024 -> 8 cores x 128.  Per core the
layout is feature-on-partition, batch-on-free (128 batch lanes in the free
dimension), so every op in the recurrence is a small [P<=128, 128] tile op.
"""

import numpy as np

import concourse.bass as bass
import concourse.mybir as mybir
import concourse.tile as tile
from concourse.bass_utils import run_bass_kernel_spmd
from concourse.vector_clock import ScopedClock

F32 = mybir.dt.float32
BF16 = mybir.dt.bfloat16
AF = mybir.ActivationFunctionType
ALU = mybir.AluOpType

BS, OBS_LEN, TS, HID = 1024, 8, 64, 64
N_CORES = 8
BSH = BS // N_CORES  # 128 batch per core


# ---------------------------------------------------------------------------
# walrus in this container rejects >1 sync-wait per instruction; split them.
class _TileContextSW(tile.TileContext):
    def _drain_and_barrier(self, tick_clock, wait_clock):
        probe = self.nc.sync.nop(hint="wait_probe", nofuse=True)
        wait_clock.add_sem_waits(
            probe.ins, ScopedClock({None: tick_clock.global_clock})
        )
        si = probe.ins.sync_info
        waits = list(si.on_wait) if si is not None and si.on_wait else []
        probe.ins.sync_info = None
        for w in waits:
            n = self.nc.sync.nop(hint="split_wait", nofuse=True)
            n.ins.sync_info = mybir.SyncInfo(on_wait=[w], on_update=[])
        self.nc.sync.drain()
        self.nc.all_engine_barrier()
        assert self.sems is not None
        popped = self.nc._tile_sem_poison_stack.pop()
        assert popped is self._sem_poison
        self.nc.clear_and_free_semaphores(list(self.sems.allocated().values()))
        self.nc.all_engine_barrier()


def _split_multi_waits(nc):
    for fn in nc.m.functions:
        for blk in fn.blocks:
            insts = blk.instructions
            out = []
            changed = False
            for inst in insts:
                si = inst.sync_info
                waits = list(si.on_wait) if si is not None and si.on_wait else []
                if len(waits) > 1:
                    for j, w in enumerate(waits[:-1]):
                        out.append(
                            mybir.InstNoOp(
                                name=f"{inst.name}-sw{j}",
                                engine=inst.engine,
                                ins=[],
                                outs=[],
                                sync_info=mybir.SyncInfo(on_wait=[w], on_update=[]),
                                bass_nofuse=True,
                            )
                        )
                    inst.sync_info = mybir.SyncInfo(
                        on_wait=[waits[-1]],
                        on_update=list(si.on_update) if si.on_update else [],
                    )
                    changed = True
                out.append(inst)
            if changed:
                insts[:] = out


# ---------------------------------------------------------------------------
# Host-side constant folding (float32 throughout, like the reference).
def _host_tables(W):
    f32 = np.float32
    sig = lambda v: (1.0 / (1.0 + np.exp(-v, dtype=np.float64))).astype(f32)
    tanh = lambda v: np.tanh(v.astype(np.float64)).astype(f32)

    b_o = W["bih_o"] + W["bhh_o"]
    b_t = W["bih_t"] + W["bhh_t"]
    Wmix_t = W["Wmix"][:, :64]
    Wmix_o = W["Wmix"][:, 64:]
    base = Wmix_o @ W["bof"] + W["bmix"]

    h = np.zeros(HID, f32)
    c = np.zeros(HID, f32)
    BC = np.zeros((32, TS), f32)
    for t in range(TS):
        g = W["Whh_t"] @ h + b_t
        i, f, gg, o = g[:64], g[64:128], g[128:192], g[192:256]
        c = sig(f) * c + sig(i) * tanh(gg)
        h = sig(o) * tanh(c)
        ts_ = W["Wtf"] @ h + W["btf"]
        BC[:, t] = W["W1"] @ (Wmix_t @ ts_ + base) + W["b1"]
    Wc = W["W1"] @ Wmix_o @ W["Wof"]

    Wt = np.concatenate([W["Whh_o"], b_o[:, None]], axis=1)  # [256, 65]
    Wx = W["Wih_o"].copy()  # [256, 2]
    Wt = Wt.copy()
    Wt[128:192] *= 2.0  # g-gate rows via tanh(x) = 2 sig(2x) - 1
    Wx[128:192] *= 2.0
    # gate-row order [f; i; o; g2]: keeps every later tensor-tensor pair on
    # matching base partitions (f/o at 0, i/g at 64).
    perm = np.concatenate([
        np.arange(64, 128), np.arange(0, 64),
        np.arange(192, 256), np.arange(128, 192)])
    Wt = Wt[perm]
    Wx = Wx[perm]

    import ml_dtypes
    bf16 = ml_dtypes.bfloat16
    return dict(
        lhsT_h=np.ascontiguousarray(Wt.T.astype(bf16)),        # [65, 256]
        lhsT_x=np.ascontiguousarray(Wx.T.astype(bf16)),        # [2, 256]
        lhsT_xw3=np.ascontiguousarray((Wx @ W["W3"]).T.astype(bf16)),  # [32, 256]
        lhsT_wc=np.ascontiguousarray(Wc.T.astype(bf16)),       # [64, 32]
        lhsT_w2=np.ascontiguousarray(W["W2"].T.astype(bf16)),  # [32, 32]
        lhsT_w3=np.ascontiguousarray(W["W3"].T.astype(bf16)),  # [32, 2]
        BCT=BC,                                     # [32, 64]
        b2c=np.ascontiguousarray(W["b2"][:, None]),  # [32, 1]
        b3=W["b3"],                                  # [2]
        ident2=np.eye(2, dtype=f32),                 # [2, 2]
    )


# ---------------------------------------------------------------------------
def _build_bass():
    nc = bass.Bass()

    din = {}
    for name, shape, dt in [
        ("obsT", [2, OBS_LEN * BSH], BF16),
        ("ys0", [2, 3 * BSH], F32),
        ("w0", [2, BSH], F32),
        ("b3b", [2, BSH], F32),
        ("lhsT_h", [65, 256], BF16),
        ("lhsT_x", [2, 256], BF16),
        ("lhsT_xw3", [32, 256], BF16),
        ("lhsT_wc", [64, 32], BF16),
        ("lhsT_w2", [32, 32], BF16),
        ("lhsT_w3", [32, 2], BF16),
        ("BCT", [32, TS], F32),
        ("b2c", [32, 1], F32),
        ("ident2", [2, 2], F32),
    ]:
        din[name] = nc.dram_tensor(name, shape, dt, kind="ExternalInput")
    y_out = nc.dram_tensor("y", [BSH, 2 * TS], F32, kind="ExternalOutput")

    with _TileContextSW(nc) as tc:
        with (
            tc.tile_pool(name="consts", bufs=1) as cp,
            tc.tile_pool(name="work", bufs=2) as wk,
            tc.tile_pool(name="pg", bufs=2, space="PSUM") as pg,
            tc.tile_pool(name="pmix", bufs=2, space="PSUM") as pmix,
            tc.tile_pool(name="pyp", bufs=1, space="PSUM") as pyp,
            tc.tile_pool(name="pout", bufs=1, space="PSUM") as pout,
        ):
            # --- constants into SBUF
            sb = {}
            for name, t in din.items():
                sb[name] = cp.tile(list(t.shape), t.dtype, tag=name, name=name)
                nc.sync.dma_start(out=sb[name], in_=t[:, :])

            hx = cp.tile([65, BSH], BF16, tag="hx")    # h (0:64) + ones row (64)
            ch = cp.tile([64, BSH], F32, tag="ch")     # c/2
            ys = cp.tile([2, (TS + 3) * BSH], F32, tag="ys")
            O_sb = cp.tile([BSH, 2 * TS], F32, tag="O_sb")

            nc.vector.memset(hx[0:64, :], 0.0)
            nc.vector.memset(hx[64:65, :], 1.0)
            nc.vector.memset(ch, 0.0)
            nc.sync.dma_start(out=ys[:, 0 : 3 * BSH], in_=din["ys0"][:, :])

            O_ps = pout.tile([BSH, 2 * TS], F32, tag="O_ps")

            lh, lx = sb["lhsT_h"], sb["lhsT_x"]
            lxw3 = sb["lhsT_xw3"]
            obsT = sb["obsT"]

            def state_update(G):
                """sigmoid(G) -> new ch (c/2) and h (hx rows 0:64, bf16)."""
                S = wk.tile([128, 256], F32, tag="S")
                nc.scalar.activation(out=S, in_=G, func=AF.Sigmoid)
                S_f, S_i = S[0:64, 0:128], S[64:128, 0:128]
                S_o, S_g = S[0:64, 128:256], S[64:128, 128:256]
                tt = wk.tile([64, BSH], F32, tag="tt")
                nc.vector.scalar_tensor_tensor(
                    out=tt, in0=S_g, scalar=0.5, in1=S_i,
                    op0=ALU.subtract, op1=ALU.mult)
                m2 = wk.tile([64, BSH], F32, tag="m2")
                nc.gpsimd.tensor_mul(out=m2, in0=S_f, in1=ch)
                nc.vector.tensor_add(out=ch, in0=tt, in1=m2)
                th = wk.tile([64, BSH], F32, tag="th")
                nc.scalar.activation(out=th, in_=ch, func=AF.Tanh, scale=2.0)
                nc.vector.tensor_mul(out=hx[0:64, :], in0=S_o, in1=th)

            # --- warmup scan over the observed trajectory
            # x-matmuls (start=True) depend only on obsT; h-matmuls close the
            # accumulation after the previous state update.
            G_cur = None
            for k in range(OBS_LEN):
                if k > 0:
                    state_update(G_cur)
                G = pg.tile([128, 256], F32, tag="G")
                x_ap = obsT[:, k * BSH : (k + 1) * BSH]
                nc.tensor.matmul(G[:, 0:128], lx[:, 0:128], x_ap,
                                 start=True, stop=False)
                nc.tensor.matmul(G[:, 128:256], lx[:, 128:256], x_ap,
                                 start=True, stop=False)
                nc.tensor.matmul(G[:, 0:128], lh[:, 0:128], hx,
                                 start=False, stop=True)
                nc.tensor.matmul(G[:, 128:256], lh[:, 128:256], hx,
                                 start=False, stop=True)
                G_cur = G

            # --- 64 output steps (software-pipelined)
            # G for step t+1 accumulates during step t:
            #   Wh~.[h_t;1] (early) + Wx.w_t (early; w_t known since t-1)
            #   + (Wx.W3).a2_t (the only gate input on the critical path)
            # since y_t = W3.a2_t + w_t  =>  Wx.y_t folds into those two terms.
            w_cur = cp.tile([2, BSH], F32, tag="w0_t")
            nc.sync.dma_start(out=w_cur, in_=din["w0"][:, :])
            wb_cur = cp.tile([2, BSH], BF16, tag="wb0_t")
            nc.vector.tensor_copy(wb_cur, w_cur)

            for t in range(TS):
                if t > 0:
                    state_update(G_cur)
                build_next = t < TS - 1
                P1 = pmix.tile([32, BSH], F32, tag="P1")
                nc.tensor.matmul(P1, sb["lhsT_wc"], hx[0:64, :],
                                 start=True, stop=True)
                if build_next:
                    G_next = pg.tile([128, 256], F32, tag="G")
                    nc.tensor.matmul(G_next[:, 0:128], lh[:, 0:128], hx,
                                     start=True, stop=False)
                    nc.tensor.matmul(G_next[:, 128:256], lh[:, 128:256], hx,
                                     start=True, stop=False)
                    nc.tensor.matmul(G_next[:, 0:128], lx[:, 0:128], wb_cur,
                                     start=False, stop=False)
                    nc.tensor.matmul(G_next[:, 128:256], lx[:, 128:256], wb_cur,
                                     start=False, stop=False)
                a1 = wk.tile([32, BSH], BF16, tag="a1")
                nc.scalar.activation(out=a1, in_=P1, func=AF.Prelu,
                                     bias=sb["BCT"][:, t : t + 1], alpha=0.1)
                P2 = pmix.tile([32, BSH], F32, tag="P2")
                nc.tensor.matmul(P2, sb["lhsT_w2"], a1, start=True, stop=True)
                a2 = wk.tile([32, BSH], BF16, tag="a2")
                nc.scalar.activation(out=a2, in_=P2, func=AF.Prelu,
                                     bias=sb["b2c"], alpha=0.1)
                if build_next:
                    nc.tensor.matmul(G_next[:, 0:128], lxw3[:, 0:128], a2,
                                     start=False, stop=True)
                    nc.tensor.matmul(G_next[:, 128:256], lxw3[:, 128:256], a2,
                                     start=False, stop=True)
                    G_cur = G_next
                PY = pyp.tile([2, BSH], F32, tag="PY")
                nc.tensor.matmul(PY, sb["lhsT_w3"], a2, start=True, stop=True)
                ys_s = ys[:, (t + 3) * BSH : (t + 4) * BSH]
                nc.vector.tensor_add(out=ys_s, in0=PY, in1=w_cur)
                # accumulate transposed output column pair (off critical path)
                nc.tensor.matmul(O_ps[:, 2 * t : 2 * t + 2], ys_s,
                                 sb["ident2"], is_transpose=True,
                                 start=True, stop=True)
                # w for step t+1 = 1.5*y_t - 0.5*y_{t-2} + b3  (off critical path)
                u = wk.tile([2, BSH], F32, tag="u")
                nc.vector.scalar_tensor_tensor(
                    out=u, in0=ys_s, scalar=1.5, in1=sb["b3b"],
                    op0=ALU.mult, op1=ALU.add)
                wn = wk.tile([2, BSH], F32, tag="wn")
                nc.vector.scalar_tensor_tensor(
                    out=wn, in0=ys[:, (t + 1) * BSH : (t + 2) * BSH],
                    scalar=-0.5, in1=u, op0=ALU.mult, op1=ALU.add)
                wb = wk.tile([2, BSH], BF16, tag="wb")
                nc.vector.tensor_copy(wb, wn)
                w_cur = wn
                wb_cur = wb

            nc.scalar.copy(out=O_sb, in_=O_ps)
            nc.sync.dma_start(out=y_out[:, :], in_=O_sb)

    _split_multi_waits(nc)
    return nc


_CACHED = {}


def _get_nc():
    if "nc" not in _CACHED:
        _CACHED["nc"] = _build_bass()
    return _CACHED["nc"]


def _in_maps(inputs):
    f32 = np.float32
    W = {k: np.ascontiguousarray(np.asarray(v), dtype=f32) for k, v in inputs.items()
         if k != "teom"}
    tabs = _host_tables(W)
    b3 = tabs.pop("b3")
    obsv = W["obsv"]  # [1024, 8, 2]

    const_part = {k: np.ascontiguousarray(v) for k, v in tabs.items()}
    const_part["b3b"] = np.ascontiguousarray(
        np.broadcast_to(b3[:, None], (2, BSH)), dtype=f32)

    maps = []
    for i in range(N_CORES):
        sh = obsv[i * BSH : (i + 1) * BSH]  # [128, 8, 2]
        m = dict(const_part)
        import ml_dtypes
        m["obsT"] = np.ascontiguousarray(
            sh.transpose(2, 1, 0).reshape(2, OBS_LEN * BSH).astype(ml_dtypes.bfloat16))
        m["ys0"] = np.ascontiguousarray(
            sh[:, 5:8, :].transpose(2, 1, 0).reshape(2, 3 * BSH))
        w0 = 1.5 * sh[:, 7, :] - 0.5 * sh[:, 5, :] + b3  # [128, 2]
        m["w0"] = np.ascontiguousarray(w0.T)
        maps.append(m)
    return maps


def run(inputs, trace=False, **kw):
    nc = _get_nc()
    res = run_bass_kernel_spmd(
        nc, _in_maps(inputs), core_ids=list(range(N_CORES)), trace=trace, **kw)
    ys = [r["y"].reshape(BSH, TS, 2) for r in res.results]
    y = np.concatenate(ys, axis=0)
    c = np.zeros((BS, TS), np.float32)
    return (y, c), res


def kernel(**inputs):
    (y, c), _ = run(inputs)
    return y, c
